# revision 2
# baseline (speedup 1.0000x reference)
"""ContextWeaver: context[i, j] = relu(sum_{k,d} node[i,k,d] * edge[j,k,d]), diag zeroed.

Strategy (8 NeuronCores, SPMD):
  - Shard node rows 8-way (1024 rows/core); replicate edge^T per core with a
    per-core column rotation of c*1024 so the diagonal block lands at local
    columns [m*128, (m+1)*128) of every 128-row strip -- the instruction
    stream is identical on all cores and diagonal masking is fully static.
  - All on-chip traffic in bf16 (rel-err budget 2e-2 >> bf16's ~4e-3):
    inputs are pre-rounded to bf16 on host, matmul accumulates fp32 in PSUM,
    relu+downcast to a bf16 strip, output DMA'd as bf16 (16 MiB/core instead
    of 32 MiB) and upconverted to fp32 on host. Halves the DMA roofline.
  - Contraction dim is 64 (= K*D); pack two independent 64-row matmuls into
    the 128x128 PE array with tile_position row tiling: partitions 0-63
    compute local columns [0, 4096), partitions 64-127 compute [4096, 8192).
  - PSUM -> SBUF relu split between ScalarE (Relu activation) and VectorE
    (tensor_scalar_max); per-strip [128, 8192] staging; 1 MiB output DMAs
    alternated across the two HWDGE rings (sync/scalar).
  - Host unshards by rotating each slab back, upcasting, and stacking.
"""

import os as _os

_os.environ.setdefault("JAX_PLATFORMS", "axon,cpu")

import ml_dtypes
import numpy as np

import concourse.bass as bass
import concourse.mybir as mybir
import concourse.tile as tile
from concourse import bacc
from concourse.bass_utils import run_bass_kernel_spmd

N = 8192          # nodes
F = 64            # contraction (K*D = 2*32)
NCORES = 8
SHARD = N // NCORES        # 1024 rows per core
HALF = N // 2              # 4096 local columns per PE row-group
MT = 128                   # output-row strip height
NT = 512                   # matmul moving free dim (one PSUM bank fp32)
DMA_CHUNK = 4096           # output DMA width (1 MiB bf16 per dma_start)
DUAL_RING = True           # alternate output DMAs across sync/scalar HWDGE rings
DVE_DUP = True             # duplicate nodeT into partitions 64-127 on-chip

F32 = mybir.dt.float32
BF16 = mybir.dt.bfloat16
NP_BF16 = ml_dtypes.bfloat16


def build_nc():
    nc = bacc.Bacc("TRN2", target_bir_lowering=False, debug=False)

    node2_d = nc.dram_tensor(
        "node2", [64 if DVE_DUP else 128, SHARD], BF16, kind="ExternalInput"
    )
    edge2_d = nc.dram_tensor("edge2", [128, HALF], BF16, kind="ExternalInput")
    mask_d = nc.dram_tensor("dmask", [128, MT], BF16, kind="ExternalInput")
    out_d = nc.dram_tensor("out", [SHARD, N], BF16, kind="ExternalOutput")

    n_strips = SHARD // MT           # 8
    n_chunks = HALF // NT            # 8 matmul pairs per strip

    with tile.TileContext(nc) as tc:
        with (
            tc.tile_pool(name="consts", bufs=1) as consts,
            tc.tile_pool(name="outp", bufs=3) as outp,
            tc.tile_pool(name="psp", bufs=4, space=bass.MemorySpace.PSUM) as psp,
        ):
            node_sb = consts.tile([128, SHARD], BF16)
            mask_sb = consts.tile([128, MT], BF16)
            edge_sb = consts.tile([128, HALF], BF16)

            # ordered so the bytes gating the first matmul pair land first:
            # edge chunk 0, node strip 0, mask, then the rest interleaved
            nodedst = node_sb[0:64, :] if DVE_DUP else node_sb[:]
            nc.sync.dma_start(out=edge_sb[:, 0:NT], in_=edge2_d[:, 0:NT])
            nc.sync.dma_start(out=nodedst[:, 0:MT], in_=node2_d[:, 0:MT])
            nc.sync.dma_start(out=mask_sb[:], in_=mask_d[:, :])
            for j in range(1, n_chunks):
                nc.sync.dma_start(
                    out=edge_sb[:, j * NT:(j + 1) * NT],
                    in_=edge2_d[:, j * NT:(j + 1) * NT],
                )
            nc.sync.dma_start(out=nodedst[:, MT:], in_=node2_d[:, MT:])
            if DVE_DUP:
                nc.vector.tensor_copy(node_sb[64:128, 0:MT], node_sb[0:64, 0:MT])
                nc.vector.tensor_copy(node_sb[64:128, MT:], node_sb[0:64, MT:])

            for m in range(n_strips):
                strip = outp.tile([128, N], BF16)
                lhs_lo = node_sb[0:64, m * MT:(m + 1) * MT]
                lhs_hi = node_sb[64:128, m * MT:(m + 1) * MT]
                for n in range(n_chunks):
                    ps_a = psp.tile([128, NT], F32)
                    ps_b = psp.tile([128, NT], F32)
                    nc.tensor.matmul(
                        ps_a[:],
                        lhs_lo,
                        edge_sb[0:64, n * NT:(n + 1) * NT],
                        start=True, stop=True,
                        tile_position=(0, 0),
                    )
                    nc.tensor.matmul(
                        ps_b[:],
                        lhs_hi,
                        edge_sb[64:128, n * NT:(n + 1) * NT],
                        start=True, stop=True,
                        tile_position=(64, 0),
                    )
                    nc.scalar.activation(
                        strip[:, n * NT:(n + 1) * NT], ps_a[:],
                        mybir.ActivationFunctionType.Relu,
                    )
                    nc.vector.tensor_scalar_max(
                        strip[:, HALF + n * NT:HALF + (n + 1) * NT], ps_b[:], 0.0,
                    )
                # zero the diagonal block (always local cols [m*MT, (m+1)*MT))
                nc.vector.tensor_mul(
                    strip[:, m * MT:(m + 1) * MT],
                    strip[:, m * MT:(m + 1) * MT],
                    mask_sb[:],
                )
                if m == 0:
                    # finer leading chunks: the first write starts after two
                    # matmul pairs instead of four, shrinking the ramp gap
                    bounds = [0, 512, 1024, 2048, 4096] + list(range(2 * DMA_CHUNK, N + 1, DMA_CHUNK))
                else:
                    bounds = list(range(0, N + 1, DMA_CHUNK))
                for q, (lo, hi) in enumerate(zip(bounds[:-1], bounds[1:])):
                    eng = nc.scalar if (DUAL_RING and q % 2 == 1) else nc.sync
                    eng.dma_start(
                        out=out_d[m * MT:(m + 1) * MT, lo:hi],
                        in_=strip[:, lo:hi],
                    )

    nc.compile()
    return nc


_NC = None


def _get_nc():
    global _NC
    if _NC is None:
        _NC = build_nc()
    return _NC


def make_in_maps(node_features: np.ndarray, edge_features: np.ndarray):
    node = np.ascontiguousarray(node_features, dtype=np.float32).reshape(N, F)
    edge = np.ascontiguousarray(edge_features, dtype=np.float32).reshape(N, F)
    node_b = node.astype(NP_BF16)
    edge_t = np.ascontiguousarray(edge.T.astype(NP_BF16))          # [64, 8192]
    mask = np.ones((128, MT), NP_BF16)
    np.fill_diagonal(mask, 0.0)

    in_maps = []
    for c in range(NCORES):
        node_t = node_b[c * SHARD:(c + 1) * SHARD].T               # [64, 1024]
        if DVE_DUP:
            node2 = np.ascontiguousarray(node_t)
        else:
            node2 = np.ascontiguousarray(np.concatenate([node_t, node_t], axis=0))
        et = np.roll(edge_t, -c * SHARD, axis=1)       # local col j' = global (j'+c*1024)%N
        edge2 = np.ascontiguousarray(np.concatenate([et[:, :HALF], et[:, HALF:]], axis=0))
        in_maps.append({"node2": node2, "edge2": edge2, "dmask": mask})
    return in_maps


def kernel(node_features: np.ndarray, edge_features: np.ndarray) -> np.ndarray:
    nc = _get_nc()
    in_maps = make_in_maps(node_features, edge_features)
    res = run_bass_kernel_spmd(nc, in_maps, core_ids=list(range(NCORES)))
    out = np.empty((N, N), np.float32)
    for c in range(NCORES):
        out[c * SHARD:(c + 1) * SHARD] = np.roll(
            res.results[c]["out"], c * SHARD, axis=1
        ).astype(np.float32)
    return out


# revision 4
# speedup vs baseline: 1.0246x; 1.0246x over previous
"""ContextWeaver: context[i, j] = relu(sum_{k,d} node[i,k,d] * edge[j,k,d]), diag zeroed.

Strategy (8 NeuronCores, SPMD):
  - Shard node rows 8-way (1024 rows/core); replicate edge^T per core with a
    per-core column rotation of c*1024 so the diagonal block lands at local
    columns [m*128, (m+1)*128) of every 128-row strip -- the instruction
    stream is identical on all cores and diagonal masking is fully static.
  - All on-chip traffic in bf16 (rel-err budget 2e-2 >> bf16's ~4e-3):
    inputs pre-rounded to bf16 on host (node^T pre-duplicated into both PE
    row-groups), matmul accumulates fp32 in PSUM, relu+downcast to a bf16
    strip, output DMA'd as bf16 (16 MiB/core) and upconverted on host.
  - Contraction dim 64 (= K*D): two independent 64-row matmuls packed into
    the 128x128 PE via tile_position row tiling; partitions 0-63 compute
    local columns [0, 4096), partitions 64-127 compute [4096, 8192).
  - PSUM pair-tiles [128,1024] (two 512-col matmuls each) so every PSUM->
    SBUF relu drain is one 1024-col op: ScalarE takes the lo half, VectorE
    the hi half, GpSimd does the (tiny) diagonal mask multiply off the
    critical path. 0.5 MiB output DMAs alternate across both HWDGE rings.
  - Host unshards by rotating each slab back, upcasting, and stacking.
"""

import os as _os

_os.environ.setdefault("JAX_PLATFORMS", "axon,cpu")

import ml_dtypes
import numpy as np

import concourse.bass as bass
import concourse.mybir as mybir
import concourse.tile as tile
from concourse import bacc
from concourse.bass_utils import run_bass_kernel_spmd

N = 8192          # nodes
F = 64            # contraction (K*D = 2*32)
NCORES = 8
SHARD = N // NCORES        # 1024 rows per core
HALF = N // 2              # 4096 local columns per PE row-group
MT = 128                   # output-row strip height
NT = 512                   # matmul moving free dim (one PSUM bank fp32)
PAIR = 2 * NT              # 1024-col drain granularity (one PSUM pair-tile)
DMA_CHUNK = 2048           # output DMA width (0.5 MiB bf16 per dma_start)

F32 = mybir.dt.float32
BF16 = mybir.dt.bfloat16
NP_BF16 = ml_dtypes.bfloat16


def build_nc():
    nc = bacc.Bacc("TRN2", target_bir_lowering=False, debug=False)

    node2_d = nc.dram_tensor("node2", [128, SHARD], BF16, kind="ExternalInput")
    edge2_d = nc.dram_tensor("edge2", [128, HALF], BF16, kind="ExternalInput")
    mask_d = nc.dram_tensor("dmask", [128, MT], BF16, kind="ExternalInput")
    out_d = nc.dram_tensor("out", [SHARD, N], BF16, kind="ExternalOutput")

    n_strips = SHARD // MT           # 8
    n_pairs = HALF // PAIR           # 4 pair-iterations per strip

    with tile.TileContext(nc) as tc:
        with (
            tc.tile_pool(name="consts", bufs=1) as consts,
            tc.tile_pool(name="outp", bufs=3) as outp,
            tc.tile_pool(name="psp", bufs=2, space=bass.MemorySpace.PSUM) as psp,
        ):
            node_sb = consts.tile([128, SHARD], BF16)
            mask_sb = consts.tile([128, MT], BF16)
            edge_sb = consts.tile([128, HALF], BF16)

            # gate the first matmul pair ASAP: its two inputs land on
            # different HWDGE rings in parallel, then the rest stream in
            nc.sync.dma_start(out=node_sb[:, 0:MT], in_=node2_d[:, 0:MT])
            nc.scalar.dma_start(out=edge_sb[:, 0:NT], in_=edge2_d[:, 0:NT])
            nc.scalar.dma_start(out=mask_sb[:], in_=mask_d[:, :])
            for j in range(1, HALF // NT):
                eng = nc.sync if j % 2 == 1 else nc.scalar
                eng.dma_start(
                    out=edge_sb[:, j * NT:(j + 1) * NT],
                    in_=edge2_d[:, j * NT:(j + 1) * NT],
                )
            nc.sync.dma_start(out=node_sb[:, MT:], in_=node2_d[:, MT:])

            for m in range(n_strips):
                strip = outp.tile([128, N], BF16)
                lhs_lo = node_sb[0:64, m * MT:(m + 1) * MT]
                lhs_hi = node_sb[64:128, m * MT:(m + 1) * MT]
                for h in range(n_pairs):
                    c0, c1 = 2 * h * NT, (2 * h + 1) * NT
                    pa = psp.tile([128, PAIR], F32)
                    nc.tensor.matmul(
                        pa[:, 0:NT], lhs_lo, edge_sb[0:64, c0:c0 + NT],
                        start=True, stop=True, tile_position=(0, 0),
                    )
                    nc.tensor.matmul(
                        pa[:, NT:PAIR], lhs_lo, edge_sb[0:64, c1:c1 + NT],
                        start=True, stop=True, tile_position=(0, 0),
                    )
                    nc.scalar.activation(
                        strip[:, h * PAIR:(h + 1) * PAIR], pa[:],
                        mybir.ActivationFunctionType.Relu,
                    )
                    pb = psp.tile([128, PAIR], F32)
                    nc.tensor.matmul(
                        pb[:, 0:NT], lhs_hi, edge_sb[64:128, c0:c0 + NT],
                        start=True, stop=True, tile_position=(64, 0),
                    )
                    nc.tensor.matmul(
                        pb[:, NT:PAIR], lhs_hi, edge_sb[64:128, c1:c1 + NT],
                        start=True, stop=True, tile_position=(64, 0),
                    )
                    nc.vector.tensor_scalar_max(
                        strip[:, HALF + h * PAIR:HALF + (h + 1) * PAIR], pb[:], 0.0,
                    )
                # zero the diagonal block (always local cols [m*MT, (m+1)*MT),
                # inside the first lo-half drain) -- GpSimd, off critical path
                nc.gpsimd.tensor_mul(
                    strip[:, m * MT:(m + 1) * MT],
                    strip[:, m * MT:(m + 1) * MT],
                    mask_sb[:],
                )
                # 4 chunks/strip; lo chunks gate on ScalarE drains, hi on
                # VectorE drains -- interleave so both rings start early
                order = [0, 2, 1, 3]          # col chunks: lo0, hi0, lo1, hi1
                for q, ci in enumerate(order):
                    lo, hi = ci * DMA_CHUNK, (ci + 1) * DMA_CHUNK
                    eng = nc.sync if (m + q) % 2 == 0 else nc.scalar
                    eng.dma_start(
                        out=out_d[m * MT:(m + 1) * MT, lo:hi],
                        in_=strip[:, lo:hi],
                    )

    nc.compile()
    return nc


_NC = None


def _get_nc():
    global _NC
    if _NC is None:
        _NC = build_nc()
    return _NC


def make_in_maps(node_features: np.ndarray, edge_features: np.ndarray):
    node = np.ascontiguousarray(node_features, dtype=np.float32).reshape(N, F)
    edge = np.ascontiguousarray(edge_features, dtype=np.float32).reshape(N, F)
    node_b = node.astype(NP_BF16)
    edge_t = np.ascontiguousarray(edge.T.astype(NP_BF16))          # [64, 8192]
    mask = np.ones((128, MT), NP_BF16)
    np.fill_diagonal(mask, 0.0)

    in_maps = []
    for c in range(NCORES):
        node_t = node_b[c * SHARD:(c + 1) * SHARD].T               # [64, 1024]
        node2 = np.ascontiguousarray(np.concatenate([node_t, node_t], axis=0))
        et = np.roll(edge_t, -c * SHARD, axis=1)       # local col j' = global (j'+c*1024)%N
        edge2 = np.ascontiguousarray(np.concatenate([et[:, :HALF], et[:, HALF:]], axis=0))
        in_maps.append({"node2": node2, "edge2": edge2, "dmask": mask})
    return in_maps


def kernel(node_features: np.ndarray, edge_features: np.ndarray) -> np.ndarray:
    nc = _get_nc()
    in_maps = make_in_maps(node_features, edge_features)
    res = run_bass_kernel_spmd(nc, in_maps, core_ids=list(range(NCORES)))
    out = np.empty((N, N), np.float32)
    for c in range(NCORES):
        out[c * SHARD:(c + 1) * SHARD] = np.roll(
            res.results[c]["out"], c * SHARD, axis=1
        ).astype(np.float32)
    return out
